# revision 16
# baseline (speedup 1.0000x reference)
"""Trainium2 Bass kernel for nn_DINLayer (DIN recommender forward pass).

Strategy (pure data parallel, 8 NeuronCores):
  - Batch (512) sharded 64 rows/core; embedding table + weights replicated.
  - The reference multiplies attention scores by mask = (visited_goods_ids == 0),
    so only sequence positions s with some mask nonzero contribute to x_inter.
    The host finds those positions (index prep only); the device gathers just
    those v_series slices and computes their scores exactly (including the
    Dice batch-norm statistics via a cross-core AllReduce). For typical inputs
    the mask is all-zero and x_inter == 0 exactly.
  - Profile embeddings (8 ids/row) are gathered on-device via indirect DMA.
  - x = [profile | x_inter] is AllGathered; every core computes the (small)
    top MLP on the full batch redundantly, so the Dice batch statistics of the
    MLP layers are local. Output is taken from core 0.

All matmul contractions run on the PE with K-splitting (K <= 128); biases are
folded as augmented ones-rows; per-channel vectors are host-replicated across
partitions; per-row statistics use ACT column bias/scale.
"""

from contextlib import ExitStack

import numpy as np

import concourse.bacc as bacc
import concourse.bass as bass
import concourse.tile as tile
from concourse import mybir
from concourse.bass_utils import run_bass_kernel_spmd
from concourse.masks import make_identity

F32 = mybir.dt.float32
I32 = mybir.dt.int32
AF = mybir.ActivationFunctionType
ALU = mybir.AluOpType
AX = mybir.AxisListType

NC = 8
B = 512
BL = B // NC          # 64 rows per core
S = 100
D = 16
V = 160000
H1, H2 = 200, 80
CA = 36               # activation-unit hidden
EPS = 1e-3


def _rep(v, p):
    """Replicate a 1-D per-channel vector across p partitions."""
    v = np.asarray(v, np.float32).reshape(1, -1)
    return np.ascontiguousarray(np.tile(v, (p, 1)))


def _host_prep(inputs):
    ids = {k: np.asarray(inputs[k]).astype(np.int32) for k in
           ["uid", "utag1", "utag2", "utag3", "utag4",
            "i_goods_id", "i_shop_id", "i_cate_id"]}
    vg = np.asarray(inputs["visited_goods_ids"]).astype(np.int32)       # [B, S]
    vs = np.asarray(inputs["visited_shop_ids"]).astype(np.int32)
    vc = np.asarray(inputs["visited_cate_ids"]).astype(np.int32)

    # mask = (vg == 0); only s-columns with any nonzero mask matter
    ss_vals = sorted(set(np.nonzero((vg == 0).any(axis=0))[0].tolist()))
    SS = len(ss_vals)

    f32 = lambda k: np.asarray(inputs[k], np.float32)
    table = np.ascontiguousarray(f32("embed_table"))                    # [V, D]

    W1 = f32("W_mlp1")                                                  # [176,200]
    w1a = np.ascontiguousarray(W1[0:128])
    w1b_aug = np.concatenate([W1[128:176], f32("b_mlp1").reshape(1, -1)], 0)
    W2m = f32("W_mlp2")                                                 # [200,80]
    w2a = np.ascontiguousarray(W2m[0:128])
    w2b_aug = np.concatenate([W2m[128:200], f32("b_mlp2").reshape(1, -1)], 0)
    woa_aug = np.concatenate([f32("W_out"), f32("b_out").reshape(1, -1)], 0)

    wrep = {
        "w1a": w1a, "w1b": np.ascontiguousarray(w1b_aug),
        "w2a": w2a, "w2b": np.ascontiguousarray(w2b_aug),
        "woa": np.ascontiguousarray(woa_aug),
        "g1r": _rep(f32("g_ln1"), 128), "be1r": _rep(f32("beta_ln1"), 128),
        "al1r": _rep(f32("alpha_mlp1"), 128),
        "g2r": _rep(f32("g_ln2"), 128), "be2r": _rep(f32("beta_ln2"), 128),
        "al2r": _rep(f32("alpha_mlp2"), 128),
        "table": table,
    }

    if SS > 0:
        Wact = f32("W_act1")                                            # [2448,36]
        Wa, Wb, Wc = Wact[0:48], Wact[48:96], Wact[96:144]
        W2 = Wact[144:].reshape(48, 48, CA)                             # [i, j, c]
        w2pp = np.empty((49, 48 * CA + CA), np.float32)
        w2pp[0:48, 0:48 * CA] = W2.transpose(1, 0, 2).reshape(48, 48 * CA)
        w2pp[48, 0:48 * CA] = (Wc - Wb).reshape(48 * CA)
        w2pp[0:48, 48 * CA:] = Wa + Wb
        w2pp[48, 48 * CA:] = f32("b_act1")
        wrep["w2pp"] = np.ascontiguousarray(w2pp)
        wrep["alactr"] = _rep(f32("alpha_act"), BL)
        wrep["waor"] = _rep(f32("W_act_out")[:, 0], BL)

    bout_val = float(np.asarray(inputs["b_act_out"], np.float32).reshape(-1)[0])

    in_maps = []
    for k in range(NC):
        lo, hi = k * BL, (k + 1) * BL
        m = dict(wrep)
        poff = np.stack([ids[n][lo:hi] for n in
                         ["uid", "utag1", "utag2", "utag3", "utag4",
                          "i_goods_id", "i_shop_id", "i_cate_id"]], axis=1)
        m["poff"] = np.ascontiguousarray(poff)                          # [BL, 8]
        if SS > 0:
            soff = np.empty((BL, 3 * SS), np.int32)
            vgsl = np.empty((BL, SS), np.int32)
            for si, s in enumerate(ss_vals):
                soff[:, 3 * si + 0] = vg[lo:hi, s]
                soff[:, 3 * si + 1] = vs[lo:hi, s]
                soff[:, 3 * si + 2] = vc[lo:hi, s]
                vgsl[:, si] = vg[lo:hi, s]
            m["soff"] = soff
            m["vgsl"] = vgsl
        in_maps.append(m)

    return SS, in_maps, bout_val


def _ln_dice_rowln(nc, sb, ps, h_ps, g_r, be_r, tag, eps_col):
    """Per-row LayerNorm of h_ps [P, N] (PSUM) -> ln tile in SBUF.

    Returns the SBUF tile ln = LN(h) * g + beta.
    """
    P, N = h_ps.shape[0], h_ps.shape[1]
    ssum = sb.tile([P, 1], F32, tag=f"{tag}_sum")
    nc.vector.tensor_reduce(out=ssum[:], in_=h_ps[:], axis=AX.X, op=ALU.add)
    scr = sb.tile([P, N], F32, tag=f"{tag}_scr")
    nc.scalar.activation(scr[:], h_ps[:], AF.Square)
    ssq = sb.tile([P, 1], F32, tag=f"{tag}_ssq")
    nc.vector.tensor_reduce(out=ssq[:], in_=scr[:], axis=AX.X, op=ALU.add)
    mu = sb.tile([P, 1], F32, tag=f"{tag}_mu")
    nc.scalar.mul(mu[:], ssum[:], 1.0 / N)
    musq = sb.tile([P, 1], F32, tag=f"{tag}_musq")
    nc.vector.tensor_tensor(out=musq[:], in0=mu[:], in1=mu[:], op=ALU.mult)
    var = sb.tile([P, 1], F32, tag=f"{tag}_var")
    nc.vector.scalar_tensor_tensor(
        out=var[:], in0=ssq[:], scalar=1.0 / N, in1=musq[:],
        op0=ALU.mult, op1=ALU.subtract)
    sd = sb.tile([P, 1], F32, tag=f"{tag}_sd")
    nc.scalar.activation(sd[:], var[:], AF.Sqrt, bias=eps_col[0:P, :], scale=1.0)
    rsq = sb.tile([P, 1], F32, tag=f"{tag}_rsq")
    nc.vector.reciprocal(rsq[:], sd[:])
    nmu = sb.tile([P, 1], F32, tag=f"{tag}_nmu")
    nc.vector.scalar_tensor_tensor(
        out=nmu[:], in0=mu[:], scalar=-1.0, in1=rsq[:],
        op0=ALU.mult, op1=ALU.mult)
    xn = sb.tile([P, N], F32, tag=f"{tag}_xn")
    nc.scalar.activation(xn[:], h_ps[:], AF.Identity, bias=nmu[:], scale=rsq[:])
    ln = sb.tile([P, N], F32, tag=f"{tag}_ln")
    nc.vector.tensor_tensor(out=ln[:], in0=xn[:], in1=g_r[:, 0:N], op=ALU.mult)
    nc.vector.tensor_tensor(out=ln[:], in0=ln[:], in1=be_r[:, 0:N], op=ALU.add)
    return ln


def _build(SS, bout_val):
    nc = bacc.Bacc("TRN2", target_bir_lowering=False, debug=False,
                   num_devices=NC)

    din = {}
    def dram_in(name, shape, dtype=F32):
        din[name] = nc.dram_tensor(name, shape, dtype, kind="ExternalInput")
        return din[name]

    table_d = dram_in("table", [V, D])
    poff_d = dram_in("poff", [BL, 8], I32)
    w1a_d = dram_in("w1a", [128, H1])
    w1b_d = dram_in("w1b", [49, H1])
    w2a_d = dram_in("w2a", [128, H2])
    w2b_d = dram_in("w2b", [73, H2])
    woa_d = dram_in("woa", [81, 2])
    g1r_d = dram_in("g1r", [128, H1])
    be1r_d = dram_in("be1r", [128, H1])
    al1r_d = dram_in("al1r", [128, H1])
    g2r_d = dram_in("g2r", [128, H2])
    be2r_d = dram_in("be2r", [128, H2])
    al2r_d = dram_in("al2r", [128, H2])
    if SS > 0:
        w2pp_d = dram_in("w2pp", [49, 48 * CA + CA])
        alact_d = dram_in("alactr", [BL, CA])
        waor_d = dram_in("waor", [BL, CA])
        soff_d = dram_in("soff", [BL, 3 * SS], I32)
        vgsl_d = dram_in("vgsl", [BL, SS], I32)

    out_d = nc.dram_tensor("out", [B, 2], F32, kind="ExternalOutput")

    rg = [list(range(NC))]
    MT = B // 128                                   # 4 m-tiles of the full batch

    with tile.TileContext(nc, num_cores=NC) as tc, ExitStack() as ctx:
        sb = ctx.enter_context(tc.tile_pool(name="sb", bufs=1))
        sb2 = ctx.enter_context(tc.tile_pool(name="sb2", bufs=2))
        ps = ctx.enter_context(tc.tile_pool(name="ps", bufs=2, space="PSUM"))
        ps1 = ctx.enter_context(tc.tile_pool(name="ps1", bufs=1, space="PSUM"))
        dram = ctx.enter_context(tc.tile_pool(name="dram", bufs=1, space="DRAM"))

        ident = sb.tile([128, 128], F32)
        make_identity(nc, ident[:])
        eps_col = sb.tile([128, 1], F32)
        nc.vector.memset(eps_col[:], EPS)

        # -------- profile gather straight into x_loc[:, 0:128] --------
        x_loc = sb.tile([BL, 176], F32)
        poff_t = sb.tile([BL, 8], I32)
        nc.sync.dma_start(out=poff_t[:], in_=poff_d.ap())
        pg = x_loc[:, 0:128]
        for f in range(8):
            nc.gpsimd.indirect_dma_start(
                out=x_loc[:, f * D:(f + 1) * D], out_offset=None,
                in_=table_d.ap(),
                in_offset=bass.IndirectOffsetOnAxis(ap=poff_t[:, f:f + 1], axis=0))

        # ---------------- x_inter [BL, 48] -> x_loc[:, 128:176] -------
        xin = x_loc[:, 128:176]
        if SS == 0:
            nc.vector.memset(xin, 0.0)
        else:
            M36 = SS * CA
            soff_t = sb.tile([BL, 3 * SS], I32)
            nc.sync.dma_start(out=soff_t[:], in_=soff_d.ap())
            vgsl_t = sb.tile([BL, SS], I32)
            nc.sync.dma_start(out=vgsl_t[:], in_=vgsl_d.ap())
            alact_t = sb.tile([BL, CA], F32)
            nc.sync.dma_start(out=alact_t[:], in_=alact_d.ap())
            waor_t = sb.tile([BL, CA], F32)
            nc.sync.dma_start(out=waor_t[:], in_=waor_d.ap())
            w2pp_t = sb.tile([49, 48 * CA + CA], F32)
            nc.sync.dma_start(out=w2pp_t[:], in_=w2pp_d.ap())

            # v_series slices, natural layout: sg [BL, SS*48]
            sg = sb.tile([BL, SS * 48], F32)
            for si in range(SS):
                for f in range(3):
                    cc = si * 48 + f * D
                    nc.gpsimd.indirect_dma_start(
                        out=sg[:, cc:cc + D], out_offset=None,
                        in_=table_d.ap(),
                        in_offset=bass.IndirectOffsetOnAxis(
                            ap=soff_t[:, 3 * si + f:3 * si + f + 1], axis=0))

            # v_item^T (augmented with ones row): viT [49, BL]
            pvT = ps.tile([48, BL], F32, tag="t128", space="PSUM")
            nc.tensor.transpose(out=pvT[:], in_=pg[:, 80:128], identity=ident[:])
            viT = sb.tile([49, BL], F32)
            nc.vector.memset(viT[:], 1.0)
            nc.any.tensor_copy(viT[0:48, :], pvT[:])

            # M_nat [BL, 1764] = viT.T @ w2pp   (N split into <=512 chunks)
            NW = 48 * CA + CA
            m_nat = sb.tile([BL, NW], F32)
            for n0 in range(0, NW, 512):
                n1 = min(n0 + 512, NW)
                pM = ps1.tile([BL, 512], F32, tag="bc", space="PSUM")
                nc.tensor.matmul(out=pM[:, 0:n1 - n0], lhsT=viT[:],
                                 rhs=w2pp_t[:, n0:n1], start=True, stop=True)
                nc.any.tensor_copy(m_nat[:, n0:n1], pM[:, 0:n1 - n0])

            # scores_pre for each slice: spre [BL, SS*36]
            spre = sb.tile([BL, M36], F32)
            for si in range(SS):
                vsl = sg[:, si * 48:(si + 1) * 48]
                prod = sb2.tile([BL, 48 * CA], F32, tag="sprod")
                nc.vector.tensor_tensor(
                    out=prod[:].rearrange("p (i c) -> p i c", c=CA),
                    in0=vsl.rearrange("p (i c) -> p i c", c=1)
                        .broadcast_to([BL, 48, CA]),
                    in1=m_nat[:, 0:48 * CA].rearrange("p (i c) -> p i c", c=CA),
                    op=ALU.mult)
                red = sb2.tile([BL, CA], F32, tag="sred")
                nc.vector.tensor_reduce(
                    out=red[:], in_=prod[:].rearrange("p (i c) -> p c i", c=CA),
                    axis=AX.X, op=ALU.add)
                nc.vector.tensor_tensor(
                    out=spre[:, si * CA:(si + 1) * CA], in0=red[:],
                    in1=m_nat[:, 48 * CA:], op=ALU.add)

            # dice-1 statistics over the full batch: AllReduce of [1, 2*M36]
            ones_c = sb.tile([BL, 1], F32)
            nc.vector.memset(ones_c[:], 1.0)
            sq1 = sb.tile([BL, M36], F32)
            nc.scalar.activation(sq1[:], spre[:], AF.Square)
            stloc = sb.tile([1, 2 * M36], F32)
            for (src, off) in ((spre, 0), (sq1, M36)):
                for n0 in range(0, M36, 512):
                    n1 = min(n0 + 512, M36)
                    pst = ps1.tile([1, 512], F32, tag="st", space="PSUM")
                    nc.tensor.matmul(out=pst[:, 0:n1 - n0], lhsT=ones_c[:],
                                     rhs=src[:, n0:n1], start=True, stop=True)
                    nc.any.tensor_copy(stloc[:, off + n0:off + n1],
                                       pst[:, 0:n1 - n0])
            ar_in = dram.tile([1, 2 * M36], F32)
            ar_out = dram.tile([1, 2 * M36], F32)
            nc.sync.dma_start(out=ar_in[:], in_=stloc[:])
            nc.gpsimd.collective_compute(
                "AllReduce", ALU.add, ins=[ar_in[:].opt()],
                outs=[ar_out[:].opt()], replica_groups=rg)
            stg = sb.tile([1, 2 * M36], F32)
            nc.sync.dma_start(out=stg[:], in_=ar_out[:])

            mu1 = sb.tile([1, M36], F32)
            nc.scalar.mul(mu1[:], stg[:, 0:M36], 1.0 / B)
            ex2 = sb.tile([1, M36], F32)
            nc.scalar.mul(ex2[:], stg[:, M36:], 1.0 / B)
            musq1 = sb.tile([1, M36], F32)
            nc.vector.tensor_tensor(out=musq1[:], in0=mu1[:], in1=mu1[:],
                                    op=ALU.mult)
            var1 = sb.tile([1, M36], F32)
            nc.vector.tensor_tensor(out=var1[:], in0=ex2[:], in1=musq1[:],
                                    op=ALU.subtract)
            sd1 = sb.tile([1, M36], F32)
            nc.scalar.activation(sd1[:], var1[:], AF.Sqrt, bias=eps_col[0:1, :],
                                 scale=1.0)
            rsq1 = sb.tile([1, M36], F32)
            nc.vector.reciprocal(rsq1[:], sd1[:])
            nmu1 = sb.tile([1, M36], F32)
            nc.vector.scalar_tensor_tensor(
                out=nmu1[:], in0=mu1[:], scalar=-1.0, in1=rsq1[:],
                op0=ALU.mult, op1=ALU.mult)
            # broadcast to BL partitions via K=1 matmul
            ones_r = sb.tile([1, 128], F32)
            nc.vector.memset(ones_r[:], 1.0)
            ab1 = sb.tile([BL, 2 * M36], F32)
            for (src, off) in ((rsq1, 0), (nmu1, M36)):
                for n0 in range(0, M36, 512):
                    n1 = min(n0 + 512, M36)
                    pbc = ps1.tile([BL, 512], F32, tag="bc", space="PSUM")
                    nc.tensor.matmul(out=pbc[:, 0:n1 - n0],
                                     lhsT=ones_r[:, 0:BL], rhs=src[:, n0:n1],
                                     start=True, stop=True)
                    nc.any.tensor_copy(ab1[:, off + n0:off + n1],
                                       pbc[:, 0:n1 - n0])

            xn1 = sb.tile([BL, M36], F32)
            nc.vector.tensor_tensor(out=xn1[:], in0=spre[:], in1=ab1[:, 0:M36],
                                    op=ALU.mult)
            nc.vector.tensor_tensor(out=xn1[:], in0=xn1[:], in1=ab1[:, M36:],
                                    op=ALU.add)
            p1 = sb.tile([BL, M36], F32)
            nc.scalar.activation(p1[:], xn1[:], AF.Sigmoid)
            omal = sb.tile([BL, CA], F32)
            nc.scalar.activation(omal[:], alact_t[:], AF.Identity,
                                 bias=1.0, scale=-1.0)
            f1 = sb.tile([BL, M36], F32)
            al3 = lambda t: t[:].rearrange("p (s c) -> p s c", s=1) \
                                .broadcast_to([BL, SS, CA])
            v3 = lambda t: t[:].rearrange("p (s c) -> p s c", c=CA)
            nc.vector.tensor_tensor(out=v3(f1), in0=v3(p1), in1=al3(omal),
                                    op=ALU.mult)
            nc.vector.tensor_tensor(out=v3(f1), in0=v3(f1), in1=al3(alact_t),
                                    op=ALU.add)
            hsc = sb.tile([BL, M36], F32)
            nc.vector.tensor_tensor(out=hsc[:], in0=spre[:], in1=f1[:],
                                    op=ALU.mult)
            nc.vector.tensor_tensor(out=v3(hsc), in0=v3(hsc), in1=al3(waor_t),
                                    op=ALU.mult)
            sc = sb.tile([BL, SS], F32)
            nc.vector.tensor_reduce(
                out=sc[:], in_=hsc[:].rearrange("p (s c) -> p s c", c=CA),
                axis=AX.X, op=ALU.add)
            msk = sb.tile([BL, SS], F32)
            nc.vector.tensor_scalar(out=msk[:], in0=vgsl_t[:], scalar1=0,
                                    scalar2=None, op0=ALU.is_equal)
            sm = sb.tile([BL, SS], F32)
            nc.vector.scalar_tensor_tensor(
                out=sm[:], in0=sc[:], scalar=bout_val, in1=msk[:],
                op0=ALU.add, op1=ALU.mult)
            # x_inter = sum_s sm[:, s] * v_slice_s
            xt = sb2.tile([BL, 48], F32, tag="xtmp")
            for si in range(SS):
                vsl = sg[:, si * 48:(si + 1) * 48]
                if si == 0:
                    nc.scalar.activation(xin, vsl, AF.Copy,
                                         scale=sm[:, 0:1])
                else:
                    nc.scalar.activation(xt[:], vsl, AF.Copy,
                                         scale=sm[:, si:si + 1])
                    nc.vector.tensor_tensor(out=xin, in0=xin, in1=xt[:],
                                            op=ALU.add)

        # ---------------- x assembly + AllGather ----------------
        ag_in = dram.tile([BL, 176], F32)
        ag_out = dram.tile([B, 176], F32)
        nc.sync.dma_start(out=ag_in[:], in_=x_loc[:])
        nc.gpsimd.collective_compute(
            "AllGather", ALU.bypass, ins=[ag_in[:].opt()],
            outs=[ag_out[:].opt()], replica_groups=rg)

        # ---------------- top MLP on the full batch (redundant) --------
        w1a_t = sb.tile([128, H1], F32)
        nc.sync.dma_start(out=w1a_t[:], in_=w1a_d.ap())
        w1b_t = sb.tile([49, H1], F32)
        nc.sync.dma_start(out=w1b_t[:], in_=w1b_d.ap())
        w2a_t = sb.tile([128, H2], F32)
        nc.sync.dma_start(out=w2a_t[:], in_=w2a_d.ap())
        w2b_t = sb.tile([73, H2], F32)
        nc.sync.dma_start(out=w2b_t[:], in_=w2b_d.ap())
        woa_t = sb.tile([81, 2], F32)
        nc.sync.dma_start(out=woa_t[:], in_=woa_d.ap())
        g1r_t = sb.tile([128, H1], F32)
        nc.sync.dma_start(out=g1r_t[:], in_=g1r_d.ap())
        be1r_t = sb.tile([128, H1], F32)
        nc.sync.dma_start(out=be1r_t[:], in_=be1r_d.ap())
        al1r_t = sb.tile([128, H1], F32)
        nc.sync.dma_start(out=al1r_t[:], in_=al1r_d.ap())
        g2r_t = sb.tile([128, H2], F32)
        nc.sync.dma_start(out=g2r_t[:], in_=g2r_d.ap())
        be2r_t = sb.tile([128, H2], F32)
        nc.sync.dma_start(out=be2r_t[:], in_=be2r_d.ap())
        al2r_t = sb.tile([128, H2], F32)
        nc.sync.dma_start(out=al2r_t[:], in_=al2r_d.ap())
        omal1 = sb.tile([128, H1], F32)
        nc.scalar.activation(omal1[:], al1r_t[:], AF.Identity,
                             bias=1.0, scale=-1.0)
        omal2 = sb.tile([128, H2], F32)
        nc.scalar.activation(omal2[:], al2r_t[:], AF.Identity,
                             bias=1.0, scale=-1.0)

        xfull = sb.tile([128, MT * 176], F32)
        nc.sync.dma_start(
            out=xfull[:].rearrange("p (t f) -> p t f", t=MT),
            in_=ag_out[:].rearrange("(t p) f -> p t f", t=MT))

        # xT tiles
        xTa = sb.tile([128, B], F32)
        xTb = sb.tile([49, B], F32)
        nc.vector.memset(xTb[:], 1.0)
        if SS == 0:
            nc.vector.memset(xTb[0:48, :], 0.0)
        for mt in range(MT):
            pT = ps.tile([128, 128], F32, tag="t128", space="PSUM")
            nc.tensor.transpose(
                out=pT[:], in_=xfull[:, mt * 176:mt * 176 + 128],
                identity=ident[:])
            nc.any.tensor_copy(xTa[:, mt * 128:(mt + 1) * 128], pT[:])
            if SS > 0:
                pTb = ps.tile([48, 128], F32, tag="t128", space="PSUM")
                nc.tensor.transpose(
                    out=pTb[:], in_=xfull[:, mt * 176 + 128:(mt + 1) * 176],
                    identity=ident[:])
                nc.any.tensor_copy(xTb[0:48, mt * 128:(mt + 1) * 128], pTb[:])

        ones_r2 = sb.tile([1, 128], F32)
        nc.vector.memset(ones_r2[:], 1.0)
        ones_c2 = sb.tile([128, 1], F32)
        nc.vector.memset(ones_c2[:], 1.0)

        def mlp_layer(xTa_, ka, wa_t, xTb_, kb, wb_t, NH, g_t, be_t, al_t,
                      omal_t, tag):
            """One Dense+LN+Dice layer over MT m-tiles. Returns list of h tiles."""
            ln_tiles = []
            ps_h = []
            for mt in range(MT):
                ph = ps.tile([128, NH], F32, tag="mm", space="PSUM")
                nc.tensor.matmul(out=ph[:], lhsT=xTa_[0:ka, mt * 128:(mt + 1) * 128],
                                 rhs=wa_t[:], start=True, stop=False)
                nc.tensor.matmul(out=ph[:], lhsT=xTb_[0:kb, mt * 128:(mt + 1) * 128],
                                 rhs=wb_t[:], start=False, stop=True)
                ln = _ln_dice_rowln(nc, sb2, ps, ph, g_t, be_t, f"{tag}{mt}",
                                    eps_col)
                ln_tiles.append(ln)
                ps_h.append(ph)
            # dice stats over full batch (local): [1, 2*NH]
            pst = ps1.tile([1, 2 * NH], F32, tag="st", space="PSUM")
            sqs = []
            for mt in range(MT):
                sq = sb2.tile([128, NH], F32, tag=f"{tag}_sq")
                nc.scalar.activation(sq[:], ln_tiles[mt][:], AF.Square)
                sqs.append(sq)
                nc.tensor.matmul(out=pst[:, 0:NH], lhsT=ones_c2[:],
                                 rhs=ln_tiles[mt][:], start=(mt == 0),
                                 stop=(mt == MT - 1))
            for mt in range(MT):
                nc.tensor.matmul(out=pst[:, NH:2 * NH], lhsT=ones_c2[:],
                                 rhs=sqs[mt][:], start=(mt == 0),
                                 stop=(mt == MT - 1))
            mu = sb.tile([1, NH], F32, tag=f"{tag}_dmu")
            nc.scalar.mul(mu[:], pst[:, 0:NH], 1.0 / B)
            ex2 = sb.tile([1, NH], F32, tag=f"{tag}_dex2")
            nc.scalar.mul(ex2[:], pst[:, NH:2 * NH], 1.0 / B)
            musq = sb.tile([1, NH], F32, tag=f"{tag}_dmusq")
            nc.vector.tensor_tensor(out=musq[:], in0=mu[:], in1=mu[:],
                                    op=ALU.mult)
            var = sb.tile([1, NH], F32, tag=f"{tag}_dvar")
            nc.vector.tensor_tensor(out=var[:], in0=ex2[:], in1=musq[:],
                                    op=ALU.subtract)
            sd = sb.tile([1, NH], F32, tag=f"{tag}_dsd")
            nc.scalar.activation(sd[:], var[:], AF.Sqrt, bias=eps_col[0:1, :],
                                 scale=1.0)
            rsq = sb.tile([1, NH], F32, tag=f"{tag}_drsq")
            nc.vector.reciprocal(rsq[:], sd[:])
            nmu = sb.tile([1, NH], F32, tag=f"{tag}_dnmu")
            nc.vector.scalar_tensor_tensor(
                out=nmu[:], in0=mu[:], scalar=-1.0, in1=rsq[:],
                op0=ALU.mult, op1=ALU.mult)
            pbc = ps1.tile([128, 2 * NH], F32, tag="bc", space="PSUM")
            nc.tensor.matmul(out=pbc[:, 0:NH], lhsT=ones_r2[:], rhs=rsq[:],
                             start=True, stop=True)
            nc.tensor.matmul(out=pbc[:, NH:2 * NH], lhsT=ones_r2[:], rhs=nmu[:],
                             start=True, stop=True)
            ab = sb.tile([128, 2 * NH], F32, tag=f"{tag}_ab")
            nc.any.tensor_copy(ab[:], pbc[:])
            h_tiles = []
            for mt in range(MT):
                ln = ln_tiles[mt]
                xn = sb2.tile([128, NH], F32, tag=f"{tag}_dxn")
                nc.vector.tensor_tensor(out=xn[:], in0=ln[:], in1=ab[:, 0:NH],
                                        op=ALU.mult)
                nc.vector.tensor_tensor(out=xn[:], in0=xn[:], in1=ab[:, NH:],
                                        op=ALU.add)
                p = sb2.tile([128, NH], F32, tag=f"{tag}_dp")
                nc.scalar.activation(p[:], xn[:], AF.Sigmoid)
                fg = sb2.tile([128, NH], F32, tag=f"{tag}_df")
                nc.vector.tensor_tensor(out=fg[:], in0=p[:], in1=omal_t[:, 0:NH],
                                        op=ALU.mult)
                nc.vector.tensor_tensor(out=fg[:], in0=fg[:], in1=al_t[:, 0:NH],
                                        op=ALU.add)
                h = sb.tile([128, NH], F32, tag=f"{tag}_h{mt}")
                nc.vector.tensor_tensor(out=h[:], in0=ln[:], in1=fg[:],
                                        op=ALU.mult)
                h_tiles.append(h)
            return h_tiles

        h1_tiles = mlp_layer(xTa, 128, w1a_t, xTb, 49, w1b_t, H1,
                             g1r_t, be1r_t, al1r_t, omal1, "L1")

        # transpose h1 -> h1Ta [128, B], h1Tb [73, B]
        h1Ta = sb.tile([128, B], F32)
        h1Tb = sb.tile([73, B], F32)
        nc.vector.memset(h1Tb[:], 1.0)
        for mt in range(MT):
            pT = ps.tile([128, 128], F32, tag="t128", space="PSUM")
            nc.tensor.transpose(out=pT[:], in_=h1_tiles[mt][:, 0:128],
                                identity=ident[:])
            nc.any.tensor_copy(h1Ta[:, mt * 128:(mt + 1) * 128], pT[:])
            pTb = ps.tile([72, 128], F32, tag="t128", space="PSUM")
            nc.tensor.transpose(out=pTb[:], in_=h1_tiles[mt][:, 128:200],
                                identity=ident[:])
            nc.any.tensor_copy(h1Tb[0:72, mt * 128:(mt + 1) * 128], pTb[:])

        h2_tiles = mlp_layer(h1Ta, 128, w2a_t, h1Tb, 73, w2b_t, H2,
                             g2r_t, be2r_t, al2r_t, omal2, "L2")

        # output layer + softmax
        h2T = sb.tile([81, B], F32)
        nc.vector.memset(h2T[:], 1.0)
        for mt in range(MT):
            pTo = ps.tile([80, 128], F32, tag="t128", space="PSUM")
            nc.tensor.transpose(out=pTo[:], in_=h2_tiles[mt][:, 0:80],
                                identity=ident[:])
            nc.any.tensor_copy(h2T[0:80, mt * 128:(mt + 1) * 128], pTo[:])

        osb = sb.tile([128, MT * 2], F32)
        for mt in range(MT):
            po = ps.tile([128, 2], F32, tag="mm", space="PSUM")
            nc.tensor.matmul(out=po[:], lhsT=h2T[:, mt * 128:(mt + 1) * 128],
                             rhs=woa_t[:], start=True, stop=True)
            mx = sb2.tile([128, 1], F32, tag="smx")
            nc.vector.tensor_reduce(out=mx[:], in_=po[:], axis=AX.X, op=ALU.max)
            nmx = sb2.tile([128, 1], F32, tag="snmx")
            nc.scalar.mul(nmx[:], mx[:], -1.0)
            ex = sb2.tile([128, 2], F32, tag="sex")
            nc.scalar.activation(ex[:], po[:], AF.Exp, bias=nmx[:], scale=1.0)
            sme = sb2.tile([128, 1], F32, tag="ssm")
            nc.vector.tensor_reduce(out=sme[:], in_=ex[:], axis=AX.X, op=ALU.add)
            rcp = sb2.tile([128, 1], F32, tag="srcp")
            nc.vector.reciprocal(rcp[:], sme[:])
            nc.scalar.activation(osb[:, mt * 2:(mt + 1) * 2], ex[:], AF.Copy,
                                 scale=rcp[:])

        nc.sync.dma_start(
            out=out_d.ap().rearrange("(t p) c -> p t c", t=MT),
            in_=osb[:].rearrange("p (t c) -> p t c", c=2))

    nc.compile()
    return nc


def kernel(**inputs) -> np.ndarray:
    SS, in_maps, bout_val = _host_prep(inputs)
    nc = _build(SS, bout_val)
    res = run_bass_kernel_spmd(nc, in_maps, core_ids=list(range(NC)))
    return res.results[0]["out"]
